# revision 15
# baseline (speedup 1.0000x reference)
"""Causal multi-head attention on 8 trn2 NeuronCores.

Sharding: data-parallel over batch (4) x tensor-parallel over head groups (2).
Core c = 2*b + g handles batch b, heads [4g, 4g+4). Each core computes its
partial output X_b-projection -> causal attention (4 heads) -> W_o slice, and
the host sums the two head-group partials per batch (the "all-reduce" of the
W_o matmul, done at zero hardware cost during unsharding).

Per-core layout trick: everything is kept "transposed" so that no on-device
transpose is ever needed:
  XT [D, S]  (host-transposed)
  QT/KT [hk, S]  via out = W.T @ XT         (PE: lhsT=W chunk, rhs=XT chunk)
  V  [S, hk]     via out = XT.T @ Wv        (plus a ones-column per head)
  S^T[k_blk, q]  via lhsT=KT slice, rhs=QT  (softmax along partitions is
                                             avoided: denominator comes from
                                             the V ones-column during AV)
  AV^T+denom     via lhsT=[V_h | 1], rhs=exp(S^T)  -> [65, q] PSUM
  out [q, D]     via lhsT = stacked normalized AV^T pairs, rhs = W_o rows
Causal masking is block-sparse: key-block J for query-block Q (512 wide) is
skipped when J*128 >= (Q+1)*512; diagonal blocks get a [128,128] triangular
additive mask and a column offset c0 so fully-masked columns are never
computed, exp'ed, or accumulated.
"""

import sys

sys.path.insert(0, "/opt/trn_rl_repo")

from contextlib import ExitStack

import numpy as np

import concourse.bass as bass
from concourse import bacc
import concourse.mybir as mybir
import concourse.tile as tile
from concourse.bass_utils import run_bass_kernel_spmd

B, S, D, H, DK = 4, 2048, 512, 8, 64
HG = 2  # head groups (tensor-parallel)
HPG = H // HG  # heads per group = 4
HKL = HPG * DK  # local projection width = 256
N_CORES = 8

F32 = mybir.dt.float32
F32R = mybir.dt.float32r
BF16 = mybir.dt.bfloat16

SQ = 512  # query block (matmul N)
SK = 128  # key block (matmul M / partition)
NEG = -1.0e9

# toggles (module-level so test.py can flip them)
MM_KIND = "bf16"  # "bf16" | "f32r" | "f32"


def _mmdt():
    return {"bf16": BF16, "f32r": F32R, "f32": F32}[MM_KIND]


def build_nc():
    nc = bacc.Bacc(None, target_bir_lowering=False, debug=False)

    MDT = _mmdt()
    xt_d = nc.declare_dram_parameter("XT", [D, S], MDT, isOutput=False)
    wq_d = nc.declare_dram_parameter("WQ", [D, HKL], MDT, isOutput=False)
    wk_d = nc.declare_dram_parameter("WK", [D, HKL], MDT, isOutput=False)
    wv_d = nc.declare_dram_parameter("WV", [D, HKL], MDT, isOutput=False)
    wo_d = nc.declare_dram_parameter("WO", [2, 128, D], MDT, isOutput=False)
    mask_d = nc.declare_dram_parameter("MASK", [128, 128], MDT, isOutput=False)
    one_d = nc.declare_dram_parameter("ONE", [128, HPG], MDT, isOutput=False)
    out_d = nc.declare_dram_parameter("OUT", [S, D], F32, isOutput=True)

    KC = D // 128  # 4 contraction chunks
    NQ = S // SQ  # 4 query blocks
    NST = S // SK  # 16 key tiles

    with tile.TileContext(nc) as tc, ExitStack() as ctx:
        pers = ctx.enter_context(tc.tile_pool(name="pers", bufs=1))
        pt_pool = ctx.enter_context(tc.tile_pool(name="pt", bufs=4))
        recip_pool = ctx.enter_context(tc.tile_pool(name="recip", bufs=4))
        rb_pool = ctx.enter_context(tc.tile_pool(name="rb", bufs=4))
        ob_pool = ctx.enter_context(tc.tile_pool(name="ob", bufs=3))
        # PSUM budget is 8 banks of [128, 512]f32; per-tag slots:
        # s01 (2 banks x 1) + av01/proj shared (2 banks x 3) = 8
        psp = ctx.enter_context(tc.tile_pool(name="psp", bufs=1, space="PSUM"))

        # ---- persistent tiles ----
        xt = [pers.tile([128, S], MDT, tag=f"xt{k}", name=f"xt{k}") for k in range(KC)]
        wq = [pers.tile([128, HKL], MDT, tag=f"wq{k}", name=f"wq{k}") for k in range(KC)]
        wk = [pers.tile([128, HKL], MDT, tag=f"wk{k}", name=f"wk{k}") for k in range(KC)]
        wv = [pers.tile([128, HKL], MDT, tag=f"wv{k}", name=f"wv{k}") for k in range(KC)]
        wo = [pers.tile([128, D], MDT, tag=f"wo{p}", name=f"wo{p}") for p in range(2)]
        mask = pers.tile([128, 128], MDT, tag="mask")
        qt = [pers.tile([128, S], MDT, tag=f"qt{t}", name=f"qt{t}") for t in range(2)]
        kt = [pers.tile([128, S], MDT, tag=f"kt{t}", name=f"kt{t}") for t in range(2)]
        vsb = [pers.tile([128, HPG, DK + 1], MDT, tag=f"v{st}", name=f"v{st}") for st in range(NST)]
        avtn = [pers.tile([128, S], MDT, tag=f"avtn{p}", name=f"avtn{p}") for p in range(2)]

        # ---- input DMAs ----
        for k in range(KC):
            nc.sync.dma_start(wq[k][:], wq_d[k * 128 : (k + 1) * 128, :])
            nc.sync.dma_start(wk[k][:], wk_d[k * 128 : (k + 1) * 128, :])
            nc.sync.dma_start(wv[k][:], wv_d[k * 128 : (k + 1) * 128, :])
        for p in range(2):
            nc.sync.dma_start(wo[p][:], wo_d[p])
        nc.sync.dma_start(mask[:], mask_d[:, :])
        for k in range(KC):
            nc.sync.dma_start(xt[k][:], xt_d[k * 128 : (k + 1) * 128, :])

        def proj_qk(t):
            # QT/KT pair-tile t: heads (2t, 2t+1) stacked on partitions
            for dst, w in ((qt, wq), (kt, wk)):
                for n in range(NQ):
                    ps = psp.tile([128, SQ], F32, tag="av01", bufs=3, name="ps")
                    for k in range(KC):
                        nc.tensor.matmul(
                            ps[:],
                            w[k][:, t * 128 : (t + 1) * 128],
                            xt[k][:, n * SQ : (n + 1) * SQ],
                            start=(k == 0),
                            stop=(k == KC - 1),
                        )
                    nc.vector.tensor_copy(dst[t][:, n * SQ : (n + 1) * SQ], ps[:])

        def proj_v():
            for st in range(NST):
                ps = psp.tile([128, HKL], F32, tag="av01", bufs=3, name="ps")
                for k in range(KC):
                    nc.tensor.matmul(
                        ps[:],
                        xt[k][:, st * 128 : (st + 1) * 128],
                        wv[k][:],
                        start=(k == 0),
                        stop=(k == KC - 1),
                    )
                nc.vector.tensor_copy(
                    vsb[st][:, :, 0:DK], ps[:].rearrange("p (h k) -> p h k", h=HPG)
                )
                nc.sync.dma_start(vsb[st][:, :, DK : DK + 1], one_d[:, :])

        def attention(p):
            # heads (2p, 2p+1).  Two causally-balanced query-block streams
            # (Q and 7-...) run interleaved so PE/ACT/DVE always have
            # independent work in flight.
            def j_step(Q, av, n_j, J):
                rel = J - 4 * Q
                c0 = rel * SK if rel >= 0 else 0
                sp = psp.tile([128, 2, SQ], F32, tag="s01", bufs=1, name="sp")
                for hh in range(2):
                    p0 = hh * 64
                    nc.tensor.matmul(
                        sp[:, hh, c0:SQ],
                        kt[p][p0 : p0 + 64, J * SK : (J + 1) * SK],
                        qt[p][p0 : p0 + 64, Q * SQ + c0 : (Q + 1) * SQ],
                        start=True,
                        stop=True,
                    )
                pt = pt_pool.tile(
                    [128, 2, SQ], _mmdt(), tag="pt", bufs=4, name="pt"
                )
                for hh in range(2):
                    nc.scalar.activation(
                        pt[:, hh, c0:SQ],
                        sp[:, hh, c0:SQ],
                        mybir.ActivationFunctionType.Exp,
                        scale=0.125,
                    )
                if rel >= 0:
                    for hh in range(2):
                        nc.vector.tensor_mul(
                            pt[:, hh, c0 : c0 + SK],
                            pt[:, hh, c0 : c0 + SK],
                            mask[:],
                        )
                for hh in range(2):
                    nc.tensor.matmul(
                        av[:, hh, c0:SQ],
                        vsb[J][:, 2 * p + hh, :],
                        pt[:, hh, c0:SQ],
                        start=(J == 0),
                        stop=(J == n_j - 1),
                    )

            def normalize(Q, av):
                for hh in range(2):
                    p0 = hh * 64
                    recip = recip_pool.tile(
                        [1, SQ], F32, tag=f"r{hh}", name=f"r{hh}"
                    )
                    nc.vector.reciprocal(recip[:], av[DK : DK + 1, hh, :])
                    rb = rb_pool.tile([64, SQ], F32, tag=f"rb{hh}", name=f"rb{hh}")
                    nc.gpsimd.partition_broadcast(rb[:], recip[:])
                    nc.vector.tensor_mul(
                        avtn[p][p0 : p0 + 64, Q * SQ : (Q + 1) * SQ],
                        av[0:DK, hh, :],
                        rb[:],
                    )

            for QA, QB in ((0, 3), (1, 2)):
                avA = psp.tile(
                    [DK + 1, 2, SQ], F32, tag="av01", bufs=3, name="avA"
                )
                avB = psp.tile(
                    [DK + 1, 2, SQ], F32, tag="av01", bufs=3, name="avB"
                )
                njA, njB = 4 * (QA + 1), 4 * (QB + 1)
                for step in range(max(njA, njB)):
                    if step < njA:
                        j_step(QA, avA, njA, step)
                    if step < njB:
                        j_step(QB, avB, njB, step)
                normalize(QA, avA)
                normalize(QB, avB)

        proj_qk(0)
        proj_qk(1)
        proj_v()
        attention(0)
        attention(1)

        # ---- output projection: OUT[q, :] = sum_p avtn[p].T @ WO[p] ----
        for m in range(S // 128):
            ps = psp.tile([128, D], F32, tag="av01", bufs=3, name="ps")
            for p in range(2):
                nc.tensor.matmul(
                    ps[:],
                    avtn[p][:, m * 128 : (m + 1) * 128],
                    wo[p][:],
                    start=(p == 0),
                    stop=(p == 1),
                )
            ob = ob_pool.tile([128, D], F32, tag="ob", name="ob")
            nc.vector.tensor_copy(ob[:], ps[:])
            nc.sync.dma_start(out_d[m * 128 : (m + 1) * 128, :], ob[:])

    nc.finalize()
    return nc


def make_mask():
    r = np.arange(128)
    return np.where(r[:, None] <= r[None, :], 1.0, 0.0).astype(np.float32)


def make_in_maps(X, W_q, W_k, W_v, W_o):
    import ml_dtypes

    mm_np = {"bf16": ml_dtypes.bfloat16, "f32r": np.float32, "f32": np.float32}[
        MM_KIND
    ]
    X = np.ascontiguousarray(np.asarray(X, dtype=np.float32))
    W_q = np.asarray(W_q, dtype=np.float32)
    W_k = np.asarray(W_k, dtype=np.float32)
    W_v = np.asarray(W_v, dtype=np.float32)
    W_o = np.asarray(W_o, dtype=np.float32)

    mask = make_mask().astype(mm_np)
    ones = np.ones((128, HPG), dtype=mm_np)
    # group g: heads [4g, 4g+4); [D, 256] with column = h_local*64 + dk
    wq_g = [
        np.ascontiguousarray(W_q[4 * g : 4 * g + 4].transpose(1, 0, 2).reshape(D, HKL))
        for g in range(HG)
    ]
    wk_g = [
        np.ascontiguousarray(W_k[4 * g : 4 * g + 4].transpose(1, 0, 2).reshape(D, HKL))
        for g in range(HG)
    ]
    wv_g = [
        np.ascontiguousarray(W_v[4 * g : 4 * g + 4].transpose(1, 0, 2).reshape(D, HKL))
        for g in range(HG)
    ]
    # W_o row for (dv, h) is dv*H + h; pair p of group g stacks heads
    # (4g+2p, 4g+2p+1): [128, D]
    wo_r = W_o.reshape(DK, H, D)
    wo_g = [
        np.ascontiguousarray(
            np.stack(
                [
                    np.concatenate(
                        [wo_r[:, 4 * g + 2 * p, :], wo_r[:, 4 * g + 2 * p + 1, :]], axis=0
                    )
                    for p in range(2)
                ]
            )
        )
        for g in range(HG)
    ]
    in_maps = []
    for c in range(N_CORES):
        b, g = divmod(c, HG)
        in_maps.append(
            {
                "XT": np.ascontiguousarray(X[b].T).astype(mm_np),
                "WQ": wq_g[g].astype(mm_np),
                "WK": wk_g[g].astype(mm_np),
                "WV": wv_g[g].astype(mm_np),
                "WO": wo_g[g].astype(mm_np),
                "MASK": mask,
                "ONE": ones,
            }
        )
    return in_maps


_NC_CACHE = {}


def get_nc():
    key = MM_KIND
    if key not in _NC_CACHE:
        _NC_CACHE[key] = build_nc()
    return _NC_CACHE[key]


def kernel(X, W_q, W_k, W_v, W_o, _trace=False, **run_kwargs):
    nc = get_nc()
    in_maps = make_in_maps(X, W_q, W_k, W_v, W_o)
    res = run_bass_kernel_spmd(
        nc, in_maps, list(range(N_CORES)), trace=_trace, **run_kwargs
    )
    out = np.empty((B, S, D), dtype=np.float32)
    for b in range(B):
        out[b] = res.results[HG * b]["OUT"] + res.results[HG * b + 1]["OUT"]
    if _trace:
        kernel.last_result = res
    return out


# revision 16
# speedup vs baseline: 1.2070x; 1.2070x over previous
"""Causal multi-head attention on 8 trn2 NeuronCores.

Sharding: data-parallel over batch (4) x tensor-parallel over head groups (2).
Core c = 2*b + g handles batch b, heads [4g, 4g+4). Each core computes its
partial output X_b-projection -> causal attention (4 heads) -> W_o slice, and
the host sums the two head-group partials per batch (the "all-reduce" of the
W_o matmul, done at zero hardware cost during unsharding).

Per-core layout trick: everything is kept "transposed" so that no on-device
transpose is ever needed:
  XT [D, S]  (host-transposed)
  QT/KT [hk, S]  via out = W.T @ XT         (PE: lhsT=W chunk, rhs=XT chunk)
  V  [S, hk]     via out = XT.T @ Wv        (plus a ones-column per head)
  S^T[k_blk, q]  via lhsT=KT slice, rhs=QT  (softmax along partitions is
                                             avoided: denominator comes from
                                             the V ones-column during AV)
  AV^T+denom     via lhsT=[V_h | 1], rhs=exp(S^T)  -> [65, q] PSUM
  out [q, D]     via lhsT = stacked normalized AV^T pairs, rhs = W_o rows
Causal masking is block-sparse: key-block J for query-block Q (512 wide) is
skipped when J*128 >= (Q+1)*512; diagonal blocks get a [128,128] triangular
additive mask and a column offset c0 so fully-masked columns are never
computed, exp'ed, or accumulated.
"""

import sys

sys.path.insert(0, "/opt/trn_rl_repo")

from contextlib import ExitStack

import numpy as np

import concourse.bass as bass
from concourse import bacc
import concourse.mybir as mybir
import concourse.tile as tile
from concourse.bass_utils import run_bass_kernel_spmd

B, S, D, H, DK = 4, 2048, 512, 8, 64
HG = 2  # head groups (tensor-parallel)
HPG = H // HG  # heads per group = 4
HKL = HPG * DK  # local projection width = 256
N_CORES = 8

F32 = mybir.dt.float32
F32R = mybir.dt.float32r
BF16 = mybir.dt.bfloat16

SQ = 512  # query block (matmul N)
SK = 128  # key block (matmul M / partition)
NEG = -1.0e9

# toggles (module-level so test.py can flip them)
MM_KIND = "bf16"  # "bf16" | "f32r" | "f32"


def _mmdt():
    return {"bf16": BF16, "f32r": F32R, "f32": F32}[MM_KIND]


def build_nc():
    nc = bacc.Bacc(None, target_bir_lowering=False, debug=False)

    MDT = _mmdt()
    xt_d = nc.declare_dram_parameter("XT", [D, S], MDT, isOutput=False)
    wq_d = nc.declare_dram_parameter("WQ", [D, HKL], MDT, isOutput=False)
    wk_d = nc.declare_dram_parameter("WK", [D, HKL], MDT, isOutput=False)
    wv_d = nc.declare_dram_parameter("WV", [D, HKL], MDT, isOutput=False)
    wo_d = nc.declare_dram_parameter("WO", [2, 128, D], MDT, isOutput=False)
    mask_d = nc.declare_dram_parameter("MASK", [128, 128], MDT, isOutput=False)
    one_d = nc.declare_dram_parameter("ONE", [128, HPG], MDT, isOutput=False)
    out_d = nc.declare_dram_parameter("OUT", [S, D], F32, isOutput=True)

    KC = D // 128  # 4 contraction chunks
    NQ = S // SQ  # 4 query blocks
    NST = S // SK  # 16 key tiles

    with tile.TileContext(nc) as tc, ExitStack() as ctx:
        pers = ctx.enter_context(tc.tile_pool(name="pers", bufs=1))
        pt_pool = ctx.enter_context(tc.tile_pool(name="pt", bufs=4))
        recip_pool = ctx.enter_context(tc.tile_pool(name="recip", bufs=4))
        rb_pool = ctx.enter_context(tc.tile_pool(name="rb", bufs=4))
        ob_pool = ctx.enter_context(tc.tile_pool(name="ob", bufs=3))
        # PSUM budget is 8 banks of [128, 512]f32; per-tag slots:
        # s01 (2 banks x 1) + av01/proj shared (2 banks x 3) = 8
        psp = ctx.enter_context(tc.tile_pool(name="psp", bufs=1, space="PSUM"))

        # ---- persistent tiles ----
        xt = [pers.tile([128, S], MDT, tag=f"xt{k}", name=f"xt{k}") for k in range(KC)]
        wq = [pers.tile([128, HKL], MDT, tag=f"wq{k}", name=f"wq{k}") for k in range(KC)]
        wk = [pers.tile([128, HKL], MDT, tag=f"wk{k}", name=f"wk{k}") for k in range(KC)]
        wv = [pers.tile([128, HKL], MDT, tag=f"wv{k}", name=f"wv{k}") for k in range(KC)]
        wo = [pers.tile([128, D], MDT, tag=f"wo{p}", name=f"wo{p}") for p in range(2)]
        mask = pers.tile([128, 128], MDT, tag="mask")
        qt = [pers.tile([128, S], MDT, tag=f"qt{t}", name=f"qt{t}") for t in range(2)]
        kt = [pers.tile([128, S], MDT, tag=f"kt{t}", name=f"kt{t}") for t in range(2)]
        vsb = [pers.tile([128, HPG, DK + 1], MDT, tag=f"v{st}", name=f"v{st}") for st in range(NST)]
        avtn = [pers.tile([128, S], MDT, tag=f"avtn{p}", name=f"avtn{p}") for p in range(2)]

        # ---- input DMAs ----
        for k in range(KC):
            nc.sync.dma_start(wq[k][:], wq_d[k * 128 : (k + 1) * 128, :])
            nc.sync.dma_start(wk[k][:], wk_d[k * 128 : (k + 1) * 128, :])
            nc.sync.dma_start(wv[k][:], wv_d[k * 128 : (k + 1) * 128, :])
        for p in range(2):
            nc.sync.dma_start(wo[p][:], wo_d[p])
        nc.sync.dma_start(mask[:], mask_d[:, :])
        for k in range(KC):
            nc.sync.dma_start(xt[k][:], xt_d[k * 128 : (k + 1) * 128, :])

        def proj_qk(t):
            # QT/KT pair-tile t: heads (2t, 2t+1) stacked on partitions
            for dst, w in ((qt, wq), (kt, wk)):
                for n in range(NQ):
                    ps = psp.tile([128, SQ], F32, tag="s01", bufs=3, name="ps")
                    for k in range(KC):
                        nc.tensor.matmul(
                            ps[:],
                            w[k][:, t * 128 : (t + 1) * 128],
                            xt[k][:, n * SQ : (n + 1) * SQ],
                            start=(k == 0),
                            stop=(k == KC - 1),
                        )
                    nc.vector.tensor_copy(dst[t][:, n * SQ : (n + 1) * SQ], ps[:])

        def proj_v():
            for st in range(NST):
                ps = psp.tile([128, HKL], F32, tag="s01", bufs=3, name="ps")
                for k in range(KC):
                    nc.tensor.matmul(
                        ps[:],
                        xt[k][:, st * 128 : (st + 1) * 128],
                        wv[k][:],
                        start=(k == 0),
                        stop=(k == KC - 1),
                    )
                nc.vector.tensor_copy(
                    vsb[st][:, :, 0:DK], ps[:].rearrange("p (h k) -> p h k", h=HPG)
                )
                nc.sync.dma_start(vsb[st][:, :, DK : DK + 1], one_d[:, :])

        def attention(p):
            # heads (2p, 2p+1).  Two causally-balanced query-block streams
            # (Q and 7-...) run interleaved so PE/ACT/DVE always have
            # independent work in flight.
            def j_step(Q, av, n_j, J):
                rel = J - 4 * Q
                c0 = rel * SK if rel >= 0 else 0
                sp = psp.tile([128, 2, SQ], F32, tag="s01", bufs=3, name="sp")
                for hh in range(2):
                    p0 = hh * 64
                    nc.tensor.matmul(
                        sp[:, hh, c0:SQ],
                        kt[p][p0 : p0 + 64, J * SK : (J + 1) * SK],
                        qt[p][p0 : p0 + 64, Q * SQ + c0 : (Q + 1) * SQ],
                        start=True,
                        stop=True,
                    )
                pt = pt_pool.tile(
                    [128, 2, SQ], _mmdt(), tag="pt", bufs=4, name="pt"
                )
                for hh in range(2):
                    nc.scalar.activation(
                        pt[:, hh, c0:SQ],
                        sp[:, hh, c0:SQ],
                        mybir.ActivationFunctionType.Exp,
                        scale=0.125,
                    )
                if rel >= 0:
                    for hh in range(2):
                        nc.vector.tensor_mul(
                            pt[:, hh, c0 : c0 + SK],
                            pt[:, hh, c0 : c0 + SK],
                            mask[:],
                        )
                for hh in range(2):
                    nc.tensor.matmul(
                        av[:, hh, c0:SQ],
                        vsb[J][:, 2 * p + hh, :],
                        pt[:, hh, c0:SQ],
                        start=(J == 0),
                        stop=(J == n_j - 1),
                    )

            def normalize(Q, av):
                for hh in range(2):
                    p0 = hh * 64
                    recip = recip_pool.tile(
                        [1, SQ], F32, tag=f"r{hh}", name=f"r{hh}"
                    )
                    nc.vector.reciprocal(recip[:], av[DK : DK + 1, hh, :])
                    rb = rb_pool.tile([64, SQ], F32, tag=f"rb{hh}", name=f"rb{hh}")
                    nc.gpsimd.partition_broadcast(rb[:], recip[:])
                    nc.vector.tensor_mul(
                        avtn[p][p0 : p0 + 64, Q * SQ : (Q + 1) * SQ],
                        av[0:DK, hh, :],
                        rb[:],
                    )

            for Q in range(NQ):
                av = psp.tile(
                    [DK + 1, 2, SQ], F32, tag="av01", bufs=1, name="av"
                )
                n_j = 4 * (Q + 1)
                for J in range(n_j):
                    j_step(Q, av, n_j, J)
                normalize(Q, av)

        proj_qk(0)
        proj_qk(1)
        proj_v()
        attention(0)
        attention(1)

        # ---- output projection: OUT[q, :] = sum_p avtn[p].T @ WO[p] ----
        for m in range(S // 128):
            ps = psp.tile([128, D], F32, tag="s01", bufs=3, name="ps")
            for p in range(2):
                nc.tensor.matmul(
                    ps[:],
                    avtn[p][:, m * 128 : (m + 1) * 128],
                    wo[p][:],
                    start=(p == 0),
                    stop=(p == 1),
                )
            ob = ob_pool.tile([128, D], F32, tag="ob", name="ob")
            nc.vector.tensor_copy(ob[:], ps[:])
            nc.sync.dma_start(out_d[m * 128 : (m + 1) * 128, :], ob[:])

    nc.finalize()
    return nc


def make_mask():
    r = np.arange(128)
    return np.where(r[:, None] <= r[None, :], 1.0, 0.0).astype(np.float32)


def make_in_maps(X, W_q, W_k, W_v, W_o):
    import ml_dtypes

    mm_np = {"bf16": ml_dtypes.bfloat16, "f32r": np.float32, "f32": np.float32}[
        MM_KIND
    ]
    X = np.ascontiguousarray(np.asarray(X, dtype=np.float32))
    W_q = np.asarray(W_q, dtype=np.float32)
    W_k = np.asarray(W_k, dtype=np.float32)
    W_v = np.asarray(W_v, dtype=np.float32)
    W_o = np.asarray(W_o, dtype=np.float32)

    mask = make_mask().astype(mm_np)
    ones = np.ones((128, HPG), dtype=mm_np)
    # group g: heads [4g, 4g+4); [D, 256] with column = h_local*64 + dk
    wq_g = [
        np.ascontiguousarray(W_q[4 * g : 4 * g + 4].transpose(1, 0, 2).reshape(D, HKL))
        for g in range(HG)
    ]
    wk_g = [
        np.ascontiguousarray(W_k[4 * g : 4 * g + 4].transpose(1, 0, 2).reshape(D, HKL))
        for g in range(HG)
    ]
    wv_g = [
        np.ascontiguousarray(W_v[4 * g : 4 * g + 4].transpose(1, 0, 2).reshape(D, HKL))
        for g in range(HG)
    ]
    # W_o row for (dv, h) is dv*H + h; pair p of group g stacks heads
    # (4g+2p, 4g+2p+1): [128, D]
    wo_r = W_o.reshape(DK, H, D)
    wo_g = [
        np.ascontiguousarray(
            np.stack(
                [
                    np.concatenate(
                        [wo_r[:, 4 * g + 2 * p, :], wo_r[:, 4 * g + 2 * p + 1, :]], axis=0
                    )
                    for p in range(2)
                ]
            )
        )
        for g in range(HG)
    ]
    in_maps = []
    for c in range(N_CORES):
        b, g = divmod(c, HG)
        in_maps.append(
            {
                "XT": np.ascontiguousarray(X[b].T).astype(mm_np),
                "WQ": wq_g[g].astype(mm_np),
                "WK": wk_g[g].astype(mm_np),
                "WV": wv_g[g].astype(mm_np),
                "WO": wo_g[g].astype(mm_np),
                "MASK": mask,
                "ONE": ones,
            }
        )
    return in_maps


_NC_CACHE = {}


def get_nc():
    key = MM_KIND
    if key not in _NC_CACHE:
        _NC_CACHE[key] = build_nc()
    return _NC_CACHE[key]


def kernel(X, W_q, W_k, W_v, W_o, _trace=False, **run_kwargs):
    nc = get_nc()
    in_maps = make_in_maps(X, W_q, W_k, W_v, W_o)
    res = run_bass_kernel_spmd(
        nc, in_maps, list(range(N_CORES)), trace=_trace, **run_kwargs
    )
    out = np.empty((B, S, D), dtype=np.float32)
    for b in range(B):
        out[b] = res.results[HG * b]["OUT"] + res.results[HG * b + 1]["OUT"]
    if _trace:
        kernel.last_result = res
    return out
